# revision 37
# baseline (speedup 1.0000x reference)
"""AttentionBlock kernel for 8 Trainium2 NeuronCores (Bass/Tile).

Problem (hardcoded shapes): x [16, 512, 32, 32] fp32, GroupNorm(32 groups,
eps=1e-5) -> 1x1-conv QKV (qkv_w [1536,512], qkv_b) -> 8-head attention over
T=1024 positions (head dim 64) -> 1x1-conv proj -> residual add.

Sharding: pure data-parallel over batch; each of the 8 cores handles 2
batches end-to-end; weights replicated; no collectives.

Per-core dataflow (per batch, all layouts channel-on-partition [128, ko, T]):
  1. GroupNorm stats per channel via bn_stats/bn_aggr, group reduction via a
     tiny constant matmul (G: [512,32] one-hot/16), rstd computed as
     exp(-0.5*ln(var+eps)) so the whole kernel needs only the
     natural_log_exp ACT table set (no per-batch table switches), broadcast
     back to channels via a second constant matmul (B = G^T one-hot), then
     tensor_scalar normalize.  norm_w/norm_b are folded into the QKV weights
     host-side, the 1/sqrt(64) attention scale and the q bias are folded into
     Wq/bq, the k bias is dropped (softmax shift invariance), and the v bias
     is folded into the proj bias.
  2. q,k = Wqk @ h as [128, T] head-pairs (head h occupies partitions
     64*(h%2)..); v^T computed directly as h^T @ Wv^T (no transposes needed).
  3. Per head: St = kz^T q in [s, t] layout (kz zero-padded to K=128 -- PE
     tiling-mode switches corrupt in-flight matmuls on this HW, so every
     matmul stays in 128-row mode), exp on ScalarE (psum->sbuf, bf16),
     AV+denominator in one matmul with lhsT = [v^T | ones] (denominator
     lands replicated on the opposite 64 partitions).  The AV psum is copied
     to SBUF immediately (frees the psum bank so the PE never stalls on the
     reciprocal chain).  Denominators are deduplicated: one [1,512] row per
     (head, half) is DMA'd into a shared [16,512] tile and 1/D is computed
     by ONE magic-seed + 2-Newton chain per 4-head group (standard
     GPSIMD/DVE ops -- the custom-DVE reciprocal misfires on this HW), the
     reciprocals bounce through a DRAM scratch tile and are broadcast back
     to 64 partitions with a stride-0 DMA, then one tensor_tensor multiply
     per head.  Attention is software-pipelined: head h's St/exp stream
     interleaves with head h-1's AV matmuls to keep the PE dense.
  4. proj matmul + (residual + proj bias) add, DMA out.  Batch b+1's
     GroupNorm+QKV is emitted before batch b's proj so the PE fills the
     softmax-tail bubble with the next batch's matmuls.
"""

import numpy as np

B, C, T = 16, 512, 1024
NH, CH = 8, 64
NG = 32
EPS = 1e-5
NCORES = 8
BPC = B // NCORES  # batches per core
KO = C // 128      # channel chunks

# --- dtype configuration -------------------------------------------------
MM_QKV = 'bf16'   # h, wqkT, wvT operand treatment (qkv + v^T matmuls)
MM_ATT = 'bf16'   # q, k, expSt, vT operand treatment (St + AV matmuls)
MM_PROJ = 'bf16'  # a, wpT operand treatment (proj matmul)
TRACE = False
DEBUG_LIGHT = False  # only h + a outputs (minimal schedule perturbation)


def _npdt(mode):
    import ml_dtypes
    return np.dtype(ml_dtypes.bfloat16) if mode == 'bf16' else np.float32


def _build_nc():
    import concourse.bass as bass
    import concourse.tile as tile
    from concourse import bacc, mybir
    from contextlib import ExitStack

    f32 = mybir.dt.float32
    f32r = mybir.dt.float32r
    bf16 = mybir.dt.bfloat16
    i32 = mybir.dt.int32

    def mmdt(mode):
        return {'bf16': bf16, 'f32r': f32r, 'f32': f32}[mode]

    dt_h = mmdt(MM_QKV)    # h tile dtype (rhs of qkv, lhsT of v^T)
    dt_att = mmdt(MM_ATT)  # q, k, expSt, vT tiles
    dt_a = mmdt(MM_PROJ)   # a tile

    nc = bacc.Bacc()
    AF = mybir.ActivationFunctionType
    ALU = mybir.AluOpType

    x_d = nc.dram_tensor("x", [BPC, 128, KO, T], bf16, kind="ExternalInput")
    wqk_d = nc.dram_tensor("wqkT", [128, KO, 2 * C], mmdt(MM_QKV), kind="ExternalInput")
    wv_d = nc.dram_tensor("wvT", [128, KO, C], mmdt(MM_QKV), kind="ExternalInput")
    wp_d = nc.dram_tensor("wpT", [128, KO, C], mmdt(MM_PROJ), kind="ExternalInput")
    bq_d = nc.dram_tensor("bq", [128, KO], f32, kind="ExternalInput")
    bp_d = nc.dram_tensor("bp", [128, KO], f32, kind="ExternalInput")
    g_d = nc.dram_tensor("gmat", [128, KO, NG], f32, kind="ExternalInput")
    b_d = nc.dram_tensor("bmat", [128, KO, 128], f32, kind="ExternalInput")
    ones_d = nc.dram_tensor("ones", [128, 64], mmdt(MM_ATT), kind="ExternalInput")
    out_d = nc.dram_tensor("out", [BPC, 128, KO, T], bf16, kind="ExternalOutput")
    if DEBUG_LIGHT:
        dbg_h = nc.dram_tensor("dbg_h", [BPC, 128, KO, T], dt_h, kind="ExternalOutput")
        dbg_a = nc.dram_tensor("dbg_a", [BPC, 128, KO, T], dt_a, kind="ExternalOutput")

    # Every matmul keeps the PE in the default 128-row tiling mode (operands
    # zero-padded to K=128 where needed).  Switching the array tiling mode
    # without a drain corrupts in-flight matmuls on this HW, and nothing in
    # this stack inserts that drain -- so we never switch.
    def mm(out, lhsT, rhs, **kw):
        assert lhsT.partition_size() == 128
        return nc.tensor.matmul(out, lhsT, rhs, **kw)

    with tile.TileContext(nc) as tc, ExitStack() as ctx:
        consts = ctx.enter_context(tc.tile_pool(name="consts", bufs=1))
        xp = ctx.enter_context(tc.tile_pool(name="xp", bufs=2))
        hp = ctx.enter_context(tc.tile_pool(name="hp", bufs=2))
        qp = ctx.enter_context(tc.tile_pool(name="qp", bufs=2))
        kzp = ctx.enter_context(tc.tile_pool(name="kzp", bufs=1))
        vtp = ctx.enter_context(tc.tile_pool(name="vtp", bufs=1))
        esp = ctx.enter_context(tc.tile_pool(name="esp", bufs=20))
        avsp = ctx.enter_context(tc.tile_pool(name="avsp", bufs=6))
        rbp = ctx.enter_context(tc.tile_pool(name="rbp", bufs=2))
        dcp = ctx.enter_context(tc.tile_pool(name="dcp", bufs=2))
        yp = ctx.enter_context(tc.tile_pool(name="yp", bufs=2))
        ap_ = ctx.enter_context(tc.tile_pool(name="ap", bufs=2))
        gnp = ctx.enter_context(tc.tile_pool(name="gnp", bufs=2))
        psS = ctx.enter_context(tc.tile_pool(name="psS", bufs=2, space="PSUM"))
        psB = ctx.enter_context(tc.tile_pool(name="psB", bufs=1, space="PSUM"))
        # Separate psum pool for GN/QKV/vT/proj so those matmuls never
        # compete with the attention St tiles (held until ScalarE's exp
        # drains them) -- lets batch b+1's QKV fill batch b's PE gaps.
        psQ = ctx.enter_context(tc.tile_pool(name="psQ", bufs=2, space="PSUM"))
        rdp = ctx.enter_context(tc.tile_pool(name="rdp", bufs=4, space="DRAM"))

        # ---- small constants first (GroupNorm needs only these + x) ----
        bq_sb = consts.tile([128, KO], f32)
        nc.sync.dma_start(bq_sb[:], bq_d[:])
        bp_sb = consts.tile([128, KO], f32)
        nc.sync.dma_start(bp_sb[:], bp_d[:])
        g_sb = consts.tile([128, KO, NG], f32)
        nc.sync.dma_start(g_sb[:], g_d[:])
        bm_sb = consts.tile([128, KO, 128], f32)
        # Magic seed constant for the Newton reciprocal (fast-inverse trick:
        # y0_bits = 0x7EF127EA - x_bits, ~5% seed error).
        magic_sb = consts.tile([128, 2], i32)
        nc.vector.memset(magic_sb[:], 0x7EF127EA)
        # Magic seed for Newton rsqrt (y0_bits = 0x5f3759df - (x_bits >> 1)).
        magic_rs = consts.tile([NG, 1], i32)
        nc.vector.memset(magic_rs[:], 0x5F3759DF)

        # kz: one zero-padded [128, T] lhsT tile per head -- head h's k on
        # partitions 64*(h%2)..+64, zeros elsewhere.  The pad rows are
        # memset ONCE; per-batch copies only touch the valid 64 rows.
        kz_sb = kzp.tile([128, NH, T], dt_att, tag="kz")
        nc.vector.memset(kz_sb[64:128, 0:NH:2, :], 0.0)
        nc.vector.memset(kz_sb[0:64, 1:NH:2, :], 0.0)

        # v^T lhsT buffer: per head-pair p the 192 columns are
        # [vT_even(64) | ones(64) | vT_odd(64)]; head 2p uses cols 0:128 of
        # the block ([vT|ones]) and head 2p+1 uses cols 64:192 ([ones|vT]).
        # The ones blocks are DMA'd once; per-batch copies only write the
        # vT columns.
        vt_sb = vtp.tile([128, 8, 4, 192], dt_att, tag="vt")
        ones_src = bass.AP(tensor=ones_d, offset=0,
                           ap=[[64, 128], [0, 32], [1, 64]])
        vt_flat = vt_sb[:].rearrange("p a b w -> p (a b) w")
        nc.sync.dma_start(vt_flat[:, :, 64:128], ones_src)

        # ---- batch 0 input before the big weights ----
        def emit_x_load(b):
            x_sb = xp.tile([128, KO, T], bf16, tag="x")
            for ko in range(KO):
                nc.sync.dma_start(x_sb[:, ko, :], x_d[b, :, ko, :])
            return x_sb

        x_tiles = [None] * BPC
        x_tiles[0] = emit_x_load(0)
        nc.sync.dma_start(bm_sb[:], b_d[:])

        wqk_sb = consts.tile([128, KO, 2 * C], mmdt(MM_QKV))
        nc.sync.dma_start(wqk_sb[:], wqk_d[:])
        wv_sb = consts.tile([128, KO, C], mmdt(MM_QKV))
        nc.sync.dma_start(wv_sb[:], wv_d[:])
        wp_sb = consts.tile([128, KO, C], mmdt(MM_PROJ))
        nc.sync.dma_start(wp_sb[:], wp_d[:])

        # per-batch live tiles
        h_tiles = [None] * BPC

        def emit_gn_qkv(b):
            """GroupNorm + QKV (q,k) + v^T for batch b."""
            x_sb = x_tiles[b]

            # ---------------- GroupNorm ----------------
            rhs3 = gnp.tile([128, KO, 3], f32, tag="rhs3")
            for ko in range(KO):
                stats = gnp.tile([128, 2, 6], f32, tag="stats")
                for j in range(2):
                    nc.vector.bn_stats(out=stats[:, j, :], in_=x_sb[:, ko, 512 * j:512 * (j + 1)])
                nc.vector.bn_aggr(out=rhs3[:, ko, 0:2], in_=stats[:])
                nc.vector.tensor_mul(rhs3[:, ko, 2:3], rhs3[:, ko, 0:1], rhs3[:, ko, 0:1])
            gps = psQ.tile([NG, 3], f32, tag="q")
            for ko in range(KO):
                mm(gps[:], g_sb[:, ko, :], rhs3[:, ko, :],
                   start=(ko == 0), stop=(ko == KO - 1))
            # var = E[var] + E[mean^2] - mean^2 ; rstd = rsqrt(var+eps) via
            # magic-seed + 2 Newton iterations on DVE (tiny [32,1] ops) --
            # keeps ScalarE's table RAM on the exp set for the whole kernel.
            gq = gnp.tile([NG, 3], f32, tag="gq")
            nc.vector.tensor_copy(gq[:], gps[:])
            gtmp = gnp.tile([NG, 2], f32, tag="gtmp")
            # [128, 2]: rows 32..127 zeroed -- they meet the zero-padded rows
            # of bmat in the broadcast matmul (rhs K must be 128).
            gst2 = gnp.tile([128, 2], f32, tag="gst2")
            nc.vector.memset(gst2[:], 0.0)
            nc.vector.tensor_copy(gst2[0:NG, 0:1], gq[:, 0:1])
            nc.vector.tensor_add(gtmp[:, 0:1], gq[:, 1:2], gq[:, 2:3])
            nc.vector.tensor_mul(gtmp[:, 1:2], gq[:, 0:1], gq[:, 0:1])
            nc.vector.tensor_sub(gtmp[:, 0:1], gtmp[:, 0:1], gtmp[:, 1:2])
            vpe = gtmp[:, 0:1]  # var (+eps below)
            nc.vector.tensor_scalar_add(vpe, vpe, EPS)
            rs = gnp.tile([NG, 3], f32, tag="rs")
            ry = rs[:, 0:1]
            ra = rs[:, 1:2]
            rb = rs[:, 2:3]
            nc.vector.tensor_scalar(          # s = x_bits >> 1
                out=ra.bitcast(i32), in0=vpe.bitcast(i32),
                scalar1=1, scalar2=None, op0=ALU.arith_shift_right)
            nc.vector.tensor_tensor(          # y0 = bits(magic - s)
                out=ry.bitcast(i32), in0=magic_rs[:],
                in1=ra.bitcast(i32), op=ALU.subtract)
            for _ in range(2):                # y = 0.5*y*(3 - x*y^2)
                nc.vector.tensor_mul(ra, vpe, ry)
                nc.vector.tensor_mul(rb, ra, ry)
                nc.vector.scalar_tensor_tensor(
                    out=ra, in0=rb, scalar=3.0, in1=ry,
                    op0=ALU.subtract, op1=ALU.mult)
                nc.vector.tensor_scalar_mul(ry, ra, -0.5)
            nc.vector.tensor_copy(gst2[0:NG, 1:2], ry)
            bst_ps = psQ.tile([128, 2 * KO], f32, tag="q")
            for ko in range(KO):
                mm(bst_ps[:, 2 * ko:2 * ko + 2], bm_sb[:, ko, :], gst2[:],
                   start=True, stop=True)
            bst = gnp.tile([128, 2 * KO], f32, tag="bst_sb")
            nc.vector.tensor_copy(bst[:], bst_ps[:])
            h_sb = hp.tile([128, KO, T], dt_h, tag="h")
            for ko in range(KO):
                nc.vector.tensor_scalar(
                    out=h_sb[:, ko, :], in0=x_sb[:, ko, :],
                    scalar1=bst[:, 2 * ko:2 * ko + 1], scalar2=bst[:, 2 * ko + 1:2 * ko + 2],
                    op0=ALU.subtract, op1=ALU.mult)
            # pre-add proj bias to residual x (x := x + bp per channel)
            for ko in range(KO):
                nc.vector.tensor_scalar(
                    out=x_sb[:, ko, :], in0=x_sb[:, ko, :],
                    scalar1=bp_sb[:, ko:ko + 1], scalar2=None, op0=ALU.add)

            h_tiles[b] = h_sb

        def emit_qkv_chunk(b, m, q_sb):
            # one 128-row output chunk of the QKV matmul; m>=4 are k chunks
            # (into kz), m<4 are q chunks (bias added).  batch 0's
            # psum->sbuf copies run on the (then idle) ScalarE so DVE
            # doesn't gate the ramp; later batches copy on DVE (ScalarE is
            # busy with the previous batch's exp stream then).
            h_sb = h_tiles[b]
            cp_copy = nc.scalar.copy if b == 0 else nc.vector.tensor_copy
            for half in range(2):
                pq = psQ.tile([128, 512], f32, tag="q")
                for ko in range(KO):
                    mm(
                        pq[:], wqk_sb[:, ko, 128 * m:128 * (m + 1)],
                        h_sb[:, ko, 512 * half:512 * (half + 1)],
                        start=(ko == 0), stop=(ko == KO - 1))
                if m < 4:
                    nc.vector.tensor_scalar(
                        out=q_sb[:, m, 512 * half:512 * (half + 1)], in0=pq[:],
                        scalar1=bq_sb[:, m:m + 1], scalar2=None, op0=ALU.add)
                else:
                    p = m - 4
                    sl = slice(512 * half, 512 * (half + 1))
                    cp_copy(kz_sb[0:64, 2 * p, sl], pq[0:64, :])
                    cp_copy(kz_sb[64:128, 2 * p + 1, sl], pq[64:128, :])

        def emit_vt(b):
            h_sb = h_tiles[b]
            cp_copy = nc.scalar.copy if b == 0 else nc.vector.tensor_copy
            for tc_i in range(8):
                pv = psQ.tile([128, 512], f32, tag="q")
                for ko in range(KO):
                    mm(
                        pv[:], h_sb[:, ko, 128 * tc_i:128 * (tc_i + 1)],
                        wv_sb[:, ko, :],
                        start=(ko == 0), stop=(ko == KO - 1))
                pvv = pv[:].rearrange("p (h c) -> p h c", c=CH)
                cp_copy(vt_sb[:, tc_i, :, 0:64], pvv[:, 0:NH:2, :])
                cp_copy(vt_sb[:, tc_i, :, 128:192], pvv[:, 1:NH:2, :])

        def emit_attn(b):
            """QKV + attention for batch b; returns a_sb."""
            q_sb = qp.tile([128, KO, T], dt_att, tag="q")

            a_sb = ap_.tile([128, KO, T], dt_a, tag="a")
            avs_list = [None] * NH
            rb_pair = [None] * (NH // 2)
            # D rows: (head h, half) -> dc[h//4] row 2*(h%4) + half
            dc_sb = [dcp.tile([8, 512], f32, tag=f"dc{g}", name=f"dc{g}")
                     for g in range(2)]

            def av_mms(avp, h_av, es_av, sc):
                p, e = h_av // 2, h_av % 2
                es = es_av[sc]
                for half in range(2):
                    mm(
                        avp[half][:], vt_sb[:, sc, p, 64 * e:64 * e + 128],
                        es[:, 512 * half:512 * (half + 1)],
                        start=(sc == 0), stop=(sc == 7))

            def finish_head(h_av, av):
                # copy av psum -> sbuf immediately (frees the psum banks) and
                # stash the denominator rows into the shared dc tile.
                e = h_av % 2
                b1 = 64 * (1 - e)
                avs = avsp.tile([128, T], f32, tag="avs")
                for half in range(2):
                    nc.vector.tensor_copy(
                        avs[:, 512 * half:512 * (half + 1)], av[half][:])
                dc = dc_sb[h_av // 4]
                r0 = 2 * (h_av % 4)
                for half in range(2):
                    nc.sync.dma_start(
                        out=dc[r0 + half:r0 + half + 1, :],
                        in_=avs[b1:b1 + 1, 512 * half:512 * (half + 1)])
                avs_list[h_av] = avs

            def emit_chain(g, eng=None, tail=False):
                # ONE magic-seed + 2-Newton reciprocal chain per 4-head
                # group, on the deduplicated [8, 512] denominator rows.
                # eng=nc.vector for the kernel-tail chain (lower latency,
                # DVE is idle then); GPSIMD otherwise (keeps DVE free).
                if eng is None:
                    eng = nc.gpsimd
                dD = dc_sb[g][:, :]
                y = yp.tile([8, 512], f32, tag="y")
                t = yp.tile([8, 512], f32, tag="t")
                eng.tensor_tensor(                # y0 = bits(magic - D_bits)
                    out=y[:].bitcast(i32),
                    in0=magic_sb[0:8, 0:1].to_broadcast((8, 512)),
                    in1=dD.bitcast(i32), op=ALU.subtract)
                eng.tensor_mul(t[:], dD, y[:])             # t = D*y0
                nc.vector.scalar_tensor_tensor(            # z1 = (t-2)*y0 = -y1
                    out=y[:], in0=t[:], scalar=2.0, in1=y[:],
                    op0=ALU.subtract, op1=ALU.mult)
                eng.tensor_mul(t[:], dD, y[:])             # t2 = D*z1 (negative)
                nc.vector.scalar_tensor_tensor(            # z2 = (t2+2)*z1 = -y2
                    out=y[:], in0=t[:], scalar=2.0, in1=y[:],
                    op0=ALU.add, op1=ALU.mult)
                nc.vector.tensor_scalar_mul(y[:], y[:], -1.0)  # r = y2 ~ 1/D
                # bounce through DRAM, then stride-0 broadcast back to the
                # 64 numerator partitions of each head.
                rd = rdp.tile([4, 1024], f32, tag="rd")
                nc.sync.dma_start(out=rd[:], in_=y[:])
                # tail chain: last pair's mults first so the split-proj
                # ko=3 matmuls unblock as early as possible
                hh_order = (2, 3, 0, 1) if tail else (0, 1, 2, 3)
                for hh in hh_order:
                    h_av = 4 * g + hh
                    p, e = h_av // 2, h_av % 2
                    b0 = 64 * e
                    if e == 0:
                        rb_pair[p] = rbp.tile([128, T], f32, tag="rb",
                                              name=f"rb{p}")
                    rB = rb_pair[p]
                    nc.sync.dma_start(
                        out=rB[b0:b0 + 64, :],
                        in_=rd[hh:hh + 1, :].to_broadcast((64, T)))
                    nc.vector.tensor_tensor(
                        out=a_sb[b0:b0 + 64, p, :],
                        in0=avs_list[h_av][b0:b0 + 64, :],
                        in1=rB[b0:b0 + 64, :], op=ALU.mult)

            # Software-pipelined attention: head h's St matmuls (PE, gated by
            # the trailing ScalarE exp stream) are interleaved with head h-1's
            # AV matmuls (PE, inputs long ready) -- keeps the PE dense and
            # warm, all in the one K=128 tiling mode.
            prev = None  # (head, es_tiles)
            for h in range(NH):
                last = h == NH - 1
                if h % 2 == 0:
                    # this pair's k and q chunks, just in time
                    emit_qkv_chunk(b, 4 + h // 2, q_sb)
                    emit_qkv_chunk(b, h // 2, q_sb)
                if h == 1:
                    # before h1's sc loop: AV(h0) matmuls emitted there read vt
                    emit_vt(b)
                avp = None
                if prev is not None:
                    avp = [psB.tile([128, 512], f32, tag=f"av{i}", name=f"av{i}")
                           for i in range(2)]
                avpL = None
                if last:
                    # the last head's AV interleaves into its OWN St/exp
                    # stream (lag 1) so the tail chain starts right after
                    # the final exp instead of 8 chunks later
                    avpL = [psB.tile([128, 512], f32, tag=f"av{i}",
                                     name=f"av{i}l") for i in range(2)]
                p = h // 2
                es_tiles = []
                for sc in range(8):
                    es = esp.tile([128, T], dt_att, tag="es")
                    st = psS.tile([128, T], f32, tag="st")
                    for half in range(2):
                        mm(
                            st[:, 512 * half:512 * (half + 1)],
                            kz_sb[:, h, 128 * sc:128 * (sc + 1)],
                            q_sb[:, p, 512 * half:512 * (half + 1)],
                            start=True, stop=True)
                    nc.scalar.activation(es[:], st[:], AF.Exp)
                    if avp is not None:
                        av_mms(avp, prev[0], prev[1], sc)
                    if last and sc > 0:
                        av_mms(avpL, h, es_tiles, sc - 1)
                    es_tiles.append(es)
                if avp is not None:
                    finish_head(prev[0], avp)
                    if prev[0] == 3:
                        emit_chain(0)
                prev = (h, es_tiles)
            av_mms(avpL, prev[0], prev[1], 7)
            finish_head(prev[0], avpL)
            emit_chain(1, eng=nc.vector if b == BPC - 1 else None,
                       tail=(b == BPC - 1))
            return a_sb

        def emit_proj(b, a_sb):
            # Residual add lands in-place in x_sb (this op is x's last
            # reader), so no separate output buffer.
            # Last batch: accumulate ko=0..2 into the residual early; the
            # ko=3 (last pair, gated by the tail reciprocal chain) term is
            # added in a second pass so the PE isn't idle during that chain.
            x_sb = x_tiles[b]
            ko_groups = [range(KO)]
            if b == BPC - 1:
                ko_groups = [range(KO - 1), range(KO - 1, KO)]
            for kos in ko_groups:
                last = kos is ko_groups[-1]
                m_order = (3, 0, 1, 2) if (last and b == BPC - 1) else range(KO)
                for m in m_order:
                    for half in range(2):
                        po = psQ.tile([128, 512], f32, tag="q")
                        for i, ko in enumerate(kos):
                            mm(
                                po[:], wp_sb[:, ko, 128 * m:128 * (m + 1)],
                                a_sb[:, ko, 512 * half:512 * (half + 1)],
                                start=(i == 0), stop=(i == len(kos) - 1))
                        nc.vector.tensor_add(
                            x_sb[:, m, 512 * half:512 * (half + 1)], po[:],
                            x_sb[:, m, 512 * half:512 * (half + 1)])
                    if last:
                        nc.sync.dma_start(out_d[b, :, m, :], x_sb[:, m, :])

        emit_gn_qkv(0)
        a_tiles = [None] * BPC
        for b in range(BPC):
            if b + 1 < BPC:
                x_tiles[b + 1] = emit_x_load(b + 1)
            a_tiles[b] = emit_attn(b)
            if DEBUG_LIGHT:
                nc.sync.dma_start(dbg_h[b], h_tiles[b][:])
                nc.sync.dma_start(dbg_a[b], a_tiles[b][:])
            if b + 1 < BPC:
                emit_gn_qkv(b + 1)
            if b > 0:
                # previous batch's proj is emitted AFTER this batch's
                # attention so the St/exp pipeline outranks it in the PE
                # order -- proj fills attention's PE gaps instead of
                # blocking the next exp stream.
                emit_proj(b - 1, a_tiles[b - 1])
        emit_proj(BPC - 1, a_tiles[BPC - 1])

    if not nc.is_finalized():
        nc.finalize()
    return nc


def _prep_inputs(x, norm_w, norm_b, qkv_w, qkv_b, proj_w, proj_b):
    """Fold norms/biases/scale into weights; reshape for the kernel layout."""
    f = np.float32
    x = np.asarray(x, f)
    nw = np.asarray(norm_w, f)
    nb = np.asarray(norm_b, f)
    qkv_w = np.asarray(qkv_w, f)
    qkv_b = np.asarray(qkv_b, f)
    proj_w = np.asarray(proj_w, f)
    proj_b = np.asarray(proj_b, f)

    Wq, Wk, Wv = qkv_w[0:C], qkv_w[C:2 * C], qkv_w[2 * C:3 * C]
    bqv, bkv, bvv = qkv_b[0:C], qkv_b[C:2 * C], qkv_b[2 * C:3 * C]
    scale = f(1.0 / np.sqrt(CH))
    Wq_e = (Wq * nw[None, :]) * scale
    bq_e = (Wq @ nb + bqv) * scale
    Wk_e = Wk * nw[None, :]          # k bias dropped (softmax shift invariance)
    Wv_e = Wv * nw[None, :]
    bv_e = Wv @ nb + bvv
    bp_e = proj_b + proj_w @ bv_e    # v bias folded into proj bias

    def chan_chunks(vec):  # [C] -> [128, KO]
        return np.ascontiguousarray(vec.reshape(KO, 128).T)

    def lhsT_chunks(wT, dtype):  # [C, M] -> [128, KO, M]
        return np.ascontiguousarray(
            wT.reshape(KO, 128, wT.shape[1]).transpose(1, 0, 2)).astype(dtype)

    wqkT = np.concatenate([Wq_e, Wk_e], axis=0).T  # [C, 1024]
    gm = np.zeros((C, NG), f)
    gm[np.arange(C), np.arange(C) // (C // NG)] = 1.0 / (C // NG)
    # bm zero-padded to 128 rows so the broadcast matmul runs at K=128
    bm = np.zeros((128, C), f)
    bm[np.arange(C) // (C // NG), np.arange(C)] = 1.0

    dqkv = _npdt(MM_QKV)
    dproj = _npdt(MM_PROJ)
    shared = {
        "wqkT": lhsT_chunks(wqkT, dqkv),
        "wvT": lhsT_chunks(Wv_e.T, dqkv),
        "wpT": lhsT_chunks(proj_w.T, dproj),
        "bq": chan_chunks(bq_e),
        "bp": chan_chunks(bp_e),
        "gmat": np.ascontiguousarray(
            gm.reshape(KO, 128, NG).transpose(1, 0, 2)),
        "bmat": np.ascontiguousarray(bm.reshape(128, KO, 128)),
        "ones": np.ones((128, 64), _npdt(MM_ATT)),
    }
    xr = x.reshape(B, C, T)
    in_maps = []
    for c in range(NCORES):
        xc = xr[c * BPC:(c + 1) * BPC].reshape(BPC, KO, 128, T).transpose(0, 2, 1, 3)
        m = dict(shared)
        m["x"] = np.ascontiguousarray(xc).astype(_npdt('bf16'))
        in_maps.append(m)
    return in_maps


def kernel(x, norm_w, norm_b, qkv_w, qkv_b, proj_w, proj_b):
    from concourse.bass_utils import run_bass_kernel_spmd

    in_maps = _prep_inputs(x, norm_w, norm_b, qkv_w, qkv_b, proj_w, proj_b)
    nc = _build_nc()
    res = run_bass_kernel_spmd(nc, in_maps, core_ids=list(range(NCORES)), trace=TRACE)
    kernel.last_results = res
    outs = []
    for c in range(NCORES):
        oc = res.results[c]["out"]  # [BPC, 128, KO, T]
        outs.append(np.asarray(oc).transpose(0, 2, 1, 3).reshape(BPC, C, T))
    full = np.concatenate(outs, axis=0).reshape(B, C, 32, 32).astype(np.float32)
    return full
